# revision 23
# baseline (speedup 1.0000x reference)
"""DLinear layer (nn_DLinearLayer) TRN2 Bass kernel.

Math (reference):
    trend[b,t,f]  = avgpool2(x)[b,t,f] = 0.5*(x[t]+x[t+1]), last: x[T-1]
    resid         = x - trend
    out[b,n,f]    = trend[:,:,f] @ trend_W[f] + trend_b[f,n]
                  + resid[:,:,f] @ residual_W[f] + residual_b[f,n]

Both trend and resid are fixed linear maps of x along t (trend = M x with
M bidiagonal, resid = (I-M) x), so the whole layer folds into ONE GEMM:

    out[:, :, f] = x[:, :, f] @ Wc[f] + (tb+rb)[f]
    Wc[f] = residual_W[f] + M^T (trend_W[f] - residual_W[f])
    (M^T D)[t] = 0.5*(D[t] + D[t-1]),  edges: t=0 -> 0.5*D[0],
                 t=T-1 -> D[T-1] + 0.5*D[T-2]

The fold runs on host (weights are read once anyway), halving both PE
work and weight DMA vs the two-GEMM formulation. The bias row is added
on host after the gather (it is all-zeros in this model). On-device
dtype is fp16 (1 PE cycle/row, half the HBM bytes of fp32r; e5m10 keeps
rel-l2 ~4e-4 at K=1024, far under the 2e-2 gate).

Sharding: feature-expert — core k owns features {2k, 2k+1}; each
feature's [B,T] x [T,N] GEMM is independent and every weight byte is
moved exactly once across the system.

Schedule (trace-derived; measured on this part):
  * Kernel body starts after a ~7.3us framework barrier; the DMA system
    ramps from ~100 to ~400 GB/s over 8..14us, so the PE's natural
    start (~11.8us, gated by the first chunk) combined with its p-state
    ramp (1.2 GHz for the first ~3us of sustained busy, 2.4 GHz after;
    ANY idle gap resets it) is already arrival-matched. Warm-up or
    earlier starts were measured to starve at c1/c2 and end up slower.
  * W streams per 256KB chunk on two HWDGE rings in consumption order;
    x's second half is deferred behind W c2 so c1/c2 arrive with margin.
  * The LAST feature's final two chunks stream h0-halves first: the
    (b,h0) psums stop while the h1 halves are still arriving, so their
    drains overlap the final matmuls and the post-matmul tail is only
    the (b,h1) pair: parallel ACT/DVE copies + parallel SP/ACT stores.
  * Early-feature drains: DVE copies, SWDGE (gpsimd) stores — they
    never touch the W rings.
"""

import numpy as np

import concourse.bass as bass
import concourse.mybir as mybir
import concourse.tile as tile
from concourse.bass_utils import run_bass_kernel_spmd

F, B, T, N = 16, 256, 1024, 1024
NCORES = 8
FL = F // NCORES          # features per core
TC = T // 128             # t chunks (contraction tiles)
NB = B // 128             # batch tiles (output partition tiles)
NH = N // 512             # output free-dim halves
HC = TC // 2
F32 = mybir.dt.float32
F16 = mybir.dt.float16


def _split_multi_waits(nc):
    """This container's walrus build accepts at most ONE sem wait per
    instruction ("Too many sync wait commands" in CoreV3Gen setupSyncWait).
    Tile emits 2+. Move excess waits onto nofuse NoOps placed immediately
    before the owning instruction on the same engine: engines execute their
    stream in order, so semantics are unchanged."""
    for fn in nc.m.functions:
        for blk in fn.blocks:
            out = []
            for inst in blk.instructions:
                si = inst.sync_info
                if si is not None and si.on_wait and len(si.on_wait) > 1:
                    waits = list(si.on_wait)
                    for j, w in enumerate(waits[:-1]):
                        out.append(mybir.InstNoOp(
                            name=f"{inst.name}-ws{j}",
                            engine=inst.engine,
                            bass_nofuse=True,
                            sync_info=mybir.SyncInfo(on_wait=[w], on_update=[]),
                        ))
                    si.on_wait = [waits[-1]]
                out.append(inst)
            blk.instructions[:] = out


def _build():
    nc = bass.Bass(trn_type="TRN2")

    x_d = nc.dram_tensor("x", [FL, 128, TC, B], F16, kind="ExternalInput")
    wc_d = nc.dram_tensor("Wc", [FL, 128, TC, N], F16, kind="ExternalInput")
    out_d = nc.dram_tensor("out", [FL, B, N], F16, kind="ExternalOutput")

    with tile.TileContext(nc) as tc:
        with (
            tc.tile_pool(name="wp", bufs=FL) as wp,
            tc.tile_pool(name="xp", bufs=FL) as xp,
            tc.tile_pool(name="obuf", bufs=FL * NB * NH) as obp,
            tc.tile_pool(name="ps", bufs=8, space="PSUM") as psp,
        ):
            # NOTE: free-running PE warm-up (absorbing the p-state ramp
            # during DMA spin-up) was tried four ways and always measured
            # SLOWER: first-chunk arrival jitters ~10.3-11.9us, and a
            # missed handoff leaves an idle gap that resets the clock to
            # 1.2 GHz for another 3us of busy time. Instead, warm-up here
            # is DATA-GATED on the tiny first x chunk (lands ~1.4us
            # before the first W chunk): if W is late the warm-ups have
            # already finished at zero cost; if W is on time the PE
            # enters the real stream with ~1.5us of ramp credit.
            ps_warm = psp.tile([128, 512], F32, tag="ps", name="ps_warm")
            xs, ws = {}, {}
            for f in range(FL):
                xs[f] = xp.tile([128, TC, B], F16, tag="x", name=f"x_{f}")
                ws[f] = wp.tile([128, TC, N], F16, tag="w", name=f"w_{f}")

            def wdma(ring, f, c0, c1, n0=0, n1=N):
                ring.dma_start(ws[f][:, c0:c1, n0:n1],
                               wc_d[f, :, c0:c1, n0:n1])

            # ---- DMA choreography (per-chunk, consumption order).
            # f0: SP: x0c0, x0c1-3, W c2, x0h1, W c4, c6;
            #     ACT: W c0, c1, c3, c5, c7
            nc.sync.dma_start(xs[0][:, 0:1, :], x_d[0, :, 0:1, :])
            nc.sync.dma_start(xs[0][:, 1:HC, :], x_d[0, :, 1:HC, :])
            wdma(nc.scalar, 0, 0, 1)
            wdma(nc.sync, 0, 2, 3)
            wdma(nc.scalar, 0, 1, 2)
            nc.sync.dma_start(xs[0][:, HC:TC, :], x_d[0, :, HC:TC, :])
            wdma(nc.scalar, 0, 3, 4)
            wdma(nc.sync, 0, 4, 5)
            wdma(nc.scalar, 0, 5, 6)
            wdma(nc.sync, 0, 6, 7)
            wdma(nc.scalar, 0, 7, 8)
            # f1 (mirrored): ACT: x1 halves + even c; SP: odd c.
            # Final two chunks stream h0-halves first so the (b,h0)
            # psums finish while h1 halves are still in flight.
            nc.scalar.dma_start(xs[1][:, 0:HC, :], x_d[1, :, 0:HC, :])
            wdma(nc.sync, 1, 0, 1)
            wdma(nc.scalar, 1, 2, 3)
            wdma(nc.sync, 1, 1, 2)
            nc.scalar.dma_start(xs[1][:, HC:TC, :], x_d[1, :, HC:TC, :])
            wdma(nc.sync, 1, 3, 4)
            wdma(nc.scalar, 1, 4, 5)
            wdma(nc.sync, 1, 5, 6)
            wdma(nc.scalar, 1, 6, 7, 0, 512)
            wdma(nc.sync, 1, 7, 8, 0, 512)
            wdma(nc.scalar, 1, 6, 7, 512, N)
            wdma(nc.sync, 1, 7, 8, 512, N)

            # Data-gated warm-up: reads only the first x chunk (64KB,
            # first on its ring), streams 256 moving rows each, results
            # discarded. PE is busy from x-arrival (~9.9us) through the
            # first W arrival, building p-state ramp credit.
            for i in range(7):
                nc.tensor.matmul(ps_warm[:, 0:256], xs[0][:, 0, 0:128],
                                 xs[0][:, 0, :], start=True, stop=True)

            # ---- PE: psum (b,h) accumulates c=0..TC-1, h-major per
            # chunk. For the last feature, the final two chunks run all
            # h0 matmuls first (h0 psums stop early -> early drains).
            for f in range(FL):
                psums = {(b, h): psp.tile([128, 512], F32, tag="ps",
                                          name=f"ps_{f}_{b}_{h}")
                         for b in range(NB) for h in range(NH)}
                tail = f == FL - 1

                def mms(c, h):
                    ns = slice(h * 512, (h + 1) * 512)
                    for b in range(NB):
                        nc.tensor.matmul(
                            psums[b, h][:],
                            xs[f][:, c, b * 128:(b + 1) * 128],
                            ws[f][:, c, ns],
                            start=(c == 0), stop=(c == TC - 1))

                if not tail:
                    for c in range(TC):
                        for h in range(NH):
                            mms(c, h)
                else:
                    for c in range(TC - 2):
                        for h in range(NH):
                            mms(c, h)
                    mms(TC - 2, 0)
                    mms(TC - 1, 0)   # stops the (b, h0) psums
                    mms(TC - 2, 1)
                    mms(TC - 1, 1)   # stops the (b, h1) psums

                ots = {(b, h): obp.tile([128, 512], F16, tag="o",
                                        name=f"o_{f}_{b}_{h}")
                       for b in range(NB) for h in range(NH)}
                if not tail:
                    # early feature: copies split ACT/DVE; stores ride
                    # the HWDGE rings — ring FIFO puts them behind every
                    # W chunk, so the weight stream is never delayed, and
                    # they land ~7us earlier than SWDGE did (measured:
                    # SWDGE store transfers at 30-32us gated the final
                    # barrier in every earlier rev).
                    for h in range(NH):
                        nc.scalar.copy(ots[0, h][:], psums[0, h][:])
                        nc.vector.tensor_scalar_mul(
                            ots[1, h][:], psums[1, h][:], 1.0)
                    for h in range(NH):
                        for b in range(NB):
                            bs = slice(b * 128, (b + 1) * 128)
                            ns = slice(h * 512, (h + 1) * 512)
                            ring = nc.sync if b == 0 else nc.scalar
                            ring.dma_start(out_d[f, bs, ns], ots[b, h][:])
                else:
                    # h0 drains overlap the h1 matmuls; final tail is the
                    # h1 pair: parallel copies, parallel stores.
                    nc.scalar.copy(ots[0, 0][:], psums[0, 0][:])
                    nc.vector.tensor_scalar_mul(
                        ots[1, 0][:], psums[1, 0][:], 1.0)
                    nc.sync.dma_start(out_d[f, 0:128, 0:512], ots[0, 0][:])
                    nc.sync.dma_start(out_d[f, 128:256, 0:512],
                                      ots[1, 0][:])
                    nc.scalar.copy(ots[0, 1][:], psums[0, 1][:])
                    nc.vector.tensor_scalar_mul(
                        ots[1, 1][:], psums[1, 1][:], 1.0)
                    nc.sync.dma_start(out_d[f, 0:128, 512:N], ots[0, 1][:])
                    nc.scalar.dma_start(out_d[f, 128:256, 512:N],
                                        ots[1, 1][:])

    _split_multi_waits(nc)
    return nc


_NC_CACHE = []


def kernel(**inputs) -> np.ndarray:
    x = np.asarray(inputs["history_in"], dtype=np.float32)     # [B, T, F]
    wt = np.asarray(inputs["trend_W"], dtype=np.float32)       # [F, T, N]
    wr = np.asarray(inputs["residual_W"], dtype=np.float32)    # [F, T, N]
    tb = np.asarray(inputs["trend_b"], dtype=np.float32)       # [F, N]
    rb = np.asarray(inputs["residual_b"], dtype=np.float32)    # [F, N]

    # fold avgpool into the weights: Wc = Wr + M^T (Wt - Wr)
    d = wt - wr
    md = np.empty_like(d)
    md[:, 0] = 0.5 * d[:, 0]
    md[:, 1:T - 1] = 0.5 * (d[:, 1:T - 1] + d[:, 0:T - 2])
    md[:, T - 1] = d[:, T - 1] + 0.5 * d[:, T - 2]
    wcomb = (wr + md).astype(np.float16)                       # [F, T, N]
    # partition-major: wpm[f, p, c, n] = wcomb[f, c*128+p, n]
    wpm = np.ascontiguousarray(
        wcomb.reshape(F, TC, 128, N).transpose(0, 2, 1, 3))    # [F,128,TC,N]

    xT = x.transpose(2, 1, 0)                                  # [F, T, B] view
    # partition-major: xpm[f, p, c, b] = xT[f, c*128+p, b]
    xpm = np.ascontiguousarray(
        xT.reshape(F, TC, 128, B).transpose(0, 2, 1, 3)).astype(np.float16)

    if not _NC_CACHE:
        _NC_CACHE.append(_build())
    nc = _NC_CACHE[0]

    in_maps = []
    for k in range(NCORES):
        sl = slice(FL * k, FL * (k + 1))
        in_maps.append({
            "x": np.ascontiguousarray(xpm[sl]),
            "Wc": np.ascontiguousarray(wpm[sl]),
        })

    res = run_bass_kernel_spmd(nc, in_maps, core_ids=list(range(NCORES)))
    full = np.concatenate([r["out"] for r in res.results], axis=0)  # [F, B, N]
    out = full.astype(np.float32).transpose(1, 2, 0)                # [B, N, F]
    out = out + (tb + rb).T[None]
    return np.ascontiguousarray(out)
